# revision 1
# baseline (speedup 1.0000x reference)
"""Trainium2 Bass/Tile kernel for a 5-block 1D-CNN stack
(ChannelNorm -> ReLU -> Conv1d(k=4..8) -> sigmoid gate -> re-pad).

Data-parallel over batch: 32 samples -> 4 per NeuronCore x 8 cores.

Per-core layout strategy:
  * activations kept in layout B: [S(partitions), C(free)] so the channel
    norm (reduce over C) is a free-dim bn_stats and the per-position
    norm / gate scalars are per-partition ACT scale/bias operands.
  * conv runs as PE matmuls contracting over C_in, which needs layout A
    [C(partitions), S(free)]; PE transposes (matmul w/ identity) convert
    B->A for the conv input and A->B for the conv output.
  * the sigmoid gate multiplier g = 1+score is per-position, constant
    over channels, so for blocks 1..4 it is folded algebraically into the
    next block's norm coefficients (scale = g*rstd', bias = -mean*g*rstd',
    with var scaled by g^2 before adding eps -- exact), costing zero full
    passes.  Only the final block materializes the gated output.

Matmuls (conv + gate + transposes) run as float32r: full PE rate at
moving-dim >= 256 with near-fp32 accuracy, fp32 PSUM accumulation.
"""

import numpy as np

import concourse.bass as bass
import concourse.mybir as mybir
import concourse.tile as tile
from concourse import bacc
from concourse.bass_utils import run_bass_kernel_spmd
from concourse.masks import make_identity

B, S, C = 32, 1024, 256
NCORES = 8
BL = B // NCORES  # batch per core
KS = [4, 5, 6, 7, 8]
PADS = {4: 1, 5: 2, 6: 2, 7: 3, 8: 3}
LS = {k: S + 2 * PADS[k] - k + 1 for k in KS}  # conv output lengths
HALO_L, HALO_R = 3, 5
XW = HALO_L + S + HALO_R  # 1032: layout-A activation width incl. zero halo
EPS = 1e-5
UNB = float(C) / float(C - 1)  # unbiased-variance correction (ddof=1)
NT = S // 128  # 8 s-tiles of 128 per sample
F32 = mybir.dt.float32
F32R = mybir.dt.float32r

_CACHED_NC = None


def _mm(nc, out, lhsT, rhs, start, stop):
    nc.tensor.matmul(out, lhsT, rhs, start=start, stop=stop)


def _build_block(nc, tc, pools, blk, k, xb_b_tiles, g_tile, w_sb, fc_sb,
                 identity, eps_sb, zr, out_dram):
    """Emit one conv block for all BL local batches.

    xb_b_tiles: list of BL SBUF tiles [128, NT, 256] (layout B, pre-gate).
    g_tile: None (block 0) or [128, BL, NT] gate multiplier from prev block.
    Returns (next_xb_tiles, next_g_tile). For the last block, writes output
    DMAs and returns (None, None).
    """
    p = PADS[k]
    L = LS[k]
    last = (blk == len(KS) - 1)
    sm, xnb_p, xna_p, ha_p, psc, pst, pshb, psy, xb_pool, out_p = pools

    g_next = sm.tile([128, BL, NT], F32, tag="g")
    next_xb = None if last else []

    for b in range(BL):
        xb_b = xb_b_tiles[b]

        # ---- channel-norm stats over C (free dim) ----
        bn6 = sm.tile([128, NT, 6], F32, tag="bn6")
        for st in range(NT):
            nc.vector.bn_stats(out=bn6[:, st, :], in_=xb_b[:, st, :])
        mv = sm.tile([128, NT, 2], F32, tag="mv")
        for st in range(NT):
            nc.vector.bn_aggr(out=mv[:, st, :], in_=bn6[:, st, :])
        mean = mv[:, :, 0:1].rearrange("p t o -> p (t o)")  # [128, NT]
        var = mv[:, :, 1:2].rearrange("p t o -> p (t o)")

        # ---- norm coefficients (gate of prev block folded in) ----
        rt = sm.tile([128, NT], F32, tag="rt")
        g2v = sm.tile([128, NT], F32, tag="g2v")
        if g_tile is None:
            # single DVE reader collapses the 8 bn_aggr deps into one
            # same-engine chain (walrus caps sync waits per instruction)
            nc.vector.tensor_copy(g2v, var)
        else:
            g_b = g_tile[:, b, :]  # [128, NT]
            nc.vector.tensor_mul(g2v, g_b, g_b)
            nc.vector.tensor_mul(g2v, g2v, var)
        # rt = sqrt(g^2 * var * UNB + eps)
        nc.scalar.activation(out=rt, in_=g2v,
                             func=mybir.ActivationFunctionType.Sqrt,
                             bias=eps_sb, scale=UNB)
        rr = sm.tile([128, NT], F32, tag="rr")
        nc.vector.reciprocal(rr, rt)
        scale_c = sm.tile([128, NT], F32, tag="scale_c")
        if g_tile is None:
            nc.vector.tensor_copy(scale_c, rr)
        else:
            nc.vector.tensor_mul(scale_c, rr, g_tile[:, b, :])
        bias_c = sm.tile([128, NT], F32, tag="bias_c")
        # bias = -(mean * scale)
        nc.vector.scalar_tensor_tensor(out=bias_c, in0=mean, scalar=-1.0,
                                       in1=scale_c,
                                       op0=mybir.AluOpType.mult,
                                       op1=mybir.AluOpType.mult)

        # ---- fused normalize + relu (layout B), then transpose to A ----
        xna_b = xna_p.tile([128, 2, XW], F32R, tag="xnA")  # [ci, s+halo]
        # f32r halo zeros via copy (memset on f32r fails the ISA check)
        nc.vector.tensor_copy(out=xna_b[:, :, 0:HALO_L],
                              in_=zr[:, :, 0:HALO_L])
        nc.vector.tensor_copy(out=xna_b[:, :, HALO_L + S:XW],
                              in_=zr[:, :, 0:HALO_R])
        for tg in range(NT // 4):  # groups of 4 s-tiles -> one psum bank
            xnb_ts = []
            for st in range(4 * tg, 4 * tg + 4):
                xnb_t = xnb_p.tile([128, C], F32R, tag="xnB")
                nc.scalar.activation(out=xnb_t, in_=xb_b[:, st, :],
                                     func=mybir.ActivationFunctionType.Relu,
                                     scale=scale_c[:, st:st + 1],
                                     bias=bias_c[:, st:st + 1])
                xnb_ts.append(xnb_t)
            for ci in range(2):
                ps = pst.tile([128, 512], F32R, tag="pst")
                for j, xnb_t in enumerate(xnb_ts):
                    nc.tensor.transpose(ps[:, j * 128:(j + 1) * 128],
                                        xnb_t[:, ci * 128:(ci + 1) * 128],
                                        identity)
                nc.vector.tensor_copy(
                    out=xna_b[:, ci, HALO_L + tg * 512:HALO_L + tg * 512 + 512],
                    in_=ps)

        # ---- conv as matmuls (contract over ci x dk), layout A out ----
        ha_b = ha_p.tile([128, 2, S], F32R, tag="hA")  # [co_chunk, s]
        for t in range(2):
            for co in range(2):
                pc = psc.tile([128, 512], F32, tag="psc")
                idx = 0
                for ci in range(2):
                    for dk in range(k):
                        base = HALO_L - p + t * 512 + dk
                        _mm(nc, pc,
                            w_sb[:, ci, co, dk, :],
                            xna_b[:, ci, base:base + 512],
                            start=(idx == 0), stop=(idx == 2 * k - 1))
                        idx += 1
                nc.scalar.copy(out=ha_b[:, co, t * 512:(t + 1) * 512],
                               in_=pc)
        if L < S:  # zero the re-pad tail (fp32r zeros via copy)
            for co in range(2):
                nc.vector.tensor_copy(out=ha_b[:, co, L:S],
                                      in_=zr[:, 0, 0:S - L])

        # ---- gate: y^T[s, cls] = h^T @ fc, s on partitions directly ----
        ya2 = sm.tile([128, NT, 2], F32, tag="ya2")
        for st in range(NT):
            py = psy.tile([128, 2], F32, tag="psy")
            for co in range(2):
                _mm(nc, py, ha_b[:, co, st * 128:(st + 1) * 128],
                    fc_sb[:, co, :], start=(co == 0), stop=(co == 1))
            nc.vector.tensor_copy(out=ya2[:, st, :], in_=py)
        ya = ya2[:, :, 0:1].rearrange("p t o -> p (t o)")  # [128, NT]
        yb = ya2[:, :, 1:2].rearrange("p t o -> p (t o)")
        smax = sm.tile([128, NT], F32, tag="smax")
        nc.vector.tensor_max(smax, ya, yb)
        ssum = sm.tile([128, NT], F32, tag="ssum")
        nc.vector.tensor_add(ssum, ya, yb)
        pre = sm.tile([128, NT], F32, tag="pre")
        # pre = 0.2*max + (ya+yb);  score = sigmoid(0.5*pre)
        nc.vector.scalar_tensor_tensor(out=pre, in0=smax, scalar=0.2,
                                       in1=ssum,
                                       op0=mybir.AluOpType.mult,
                                       op1=mybir.AluOpType.add)
        sc = sm.tile([128, NT], F32, tag="sc")
        nc.scalar.activation(out=sc, in_=pre,
                             func=mybir.ActivationFunctionType.Sigmoid,
                             scale=0.5)
        nc.vector.tensor_scalar_add(g_next[:, b, :], sc, 1.0)

        # ---- transpose conv output back to layout B ----
        if last:
            dst_t = out_p.tile([128, NT, C], F32, tag="osb")
        else:
            dst_t = xb_pool.tile([128, NT, C], F32, tag="xB")
            next_xb.append(dst_t)
        for st in range(NT):
            ph = pshb.tile([128, C], F32R, tag="pshb")
            for co in range(2):
                nc.tensor.transpose(ph[:, co * 128:(co + 1) * 128],
                                    ha_b[:, co, st * 128:(st + 1) * 128],
                                    identity)
            if last:
                nc.vector.tensor_scalar_mul(out=dst_t[:, st, :],
                                            in0=ph,
                                            scalar1=g_next[:, b, st:st + 1])
            else:
                nc.vector.tensor_copy(out=dst_t[:, st, :], in_=ph)

        if last:
            dst = out_dram[b].rearrange("(t p) c -> p t c", p=128)
            nc.sync.dma_start(out=dst, in_=dst_t)

    return next_xb, g_next


def _build():
    nc = bacc.Bacc("TRN2", target_bir_lowering=False, debug=False,
                   num_devices=NCORES)
    x_in = nc.dram_tensor("x", [BL, S, C], F32, kind="ExternalInput").ap()
    w_in = {k: nc.dram_tensor(f"w{k}", [128, 2, 2, k, 128], F32R,
                              kind="ExternalInput").ap() for k in KS}
    fc_in = nc.dram_tensor("fc", [128, 2, 2], F32R,
                           kind="ExternalInput").ap()
    out_dram = nc.dram_tensor("out", [BL, S, C], F32,
                              kind="ExternalOutput").ap()

    from contextlib import ExitStack
    with tile.TileContext(nc) as tc, ExitStack() as ctx:
        consts = ctx.enter_context(tc.tile_pool(name="consts", bufs=1))
        wpool = ctx.enter_context(tc.tile_pool(name="wpool", bufs=2))
        xb_pool = ctx.enter_context(tc.tile_pool(name="xb", bufs=6))
        xnb_p = ctx.enter_context(tc.tile_pool(name="xnb", bufs=6))
        xna_p = ctx.enter_context(tc.tile_pool(name="xna", bufs=4))
        ha_p = ctx.enter_context(tc.tile_pool(name="ha", bufs=4))
        sm = ctx.enter_context(tc.tile_pool(name="small", bufs=8))
        out_p = ctx.enter_context(tc.tile_pool(name="outp", bufs=2))
        psc = ctx.enter_context(tc.tile_pool(name="psc", bufs=2, space="PSUM"))
        pst = ctx.enter_context(tc.tile_pool(name="pst", bufs=2, space="PSUM"))
        pshb = ctx.enter_context(tc.tile_pool(name="pshb", bufs=2,
                                              space="PSUM"))
        psy = ctx.enter_context(tc.tile_pool(name="psy", bufs=2, space="PSUM"))

        identity_f = consts.tile([128, 128], F32)
        make_identity(nc, identity_f)
        identity = consts.tile([128, 128], F32R)
        nc.vector.tensor_copy(out=identity, in_=identity_f)
        zr_f = consts.tile([128, 2, 8], F32)
        nc.vector.memset(zr_f, 0.0)
        zr = consts.tile([128, 2, 8], F32R)
        nc.vector.tensor_copy(out=zr, in_=zr_f)
        fc_sb = consts.tile([128, 2, 2], F32R)
        nc.sync.dma_start(out=fc_sb, in_=fc_in)
        eps_sb = consts.tile([128, 1], F32)
        nc.vector.memset(eps_sb, EPS)

        # initial load: [BL, S, C] -> per-batch layout-B tiles
        xb_tiles = []
        for b in range(BL):
            t = xb_pool.tile([128, NT, C], F32, tag="xB")
            nc.sync.dma_start(out=t,
                              in_=x_in[b].rearrange("(t p) c -> p t c", p=128))
            xb_tiles.append(t)

        pools = (sm, xnb_p, xna_p, ha_p, psc, pst, pshb, psy,
                 xb_pool, out_p)
        g_tile = None
        for blk, k in enumerate(KS):
            w_sb = wpool.tile([128, 2, 2, k, 128], F32R, tag="w")
            nc.sync.dma_start(out=w_sb, in_=w_in[k])
            xb_tiles, g_tile = _build_block(
                nc, tc, pools, blk, k, xb_tiles, g_tile, w_sb, fc_sb,
                identity, eps_sb, zr, out_dram)

    nc.compile()
    return nc


def _get_nc():
    global _CACHED_NC
    if _CACHED_NC is None:
        _CACHED_NC = _build()
    return _CACHED_NC


def _prep_weights(inputs):
    """Host-side packing of conv / fc weights into the DRAM layouts."""
    arrs = {}
    for k in KS:
        W = np.asarray(inputs[f"W{k}"], np.float32)  # [co, ci, k]
        Wt = W.transpose(1, 0, 2)                    # [ci, co, k]
        Wt = Wt.reshape(2, 128, 2, 128, k)           # [ci_ch, ci_in, co_ch, co_in, k]
        Wt = Wt.transpose(1, 0, 2, 4, 3)             # [ci_in, ci_ch, co_ch, k, co_in]
        arrs[f"w{k}"] = np.ascontiguousarray(Wt, np.float32)
    fc = np.asarray(inputs["fc_w"], np.float32)[:, :, 0]  # [2, co=256]
    fcT = fc.T.reshape(2, 128, 2).transpose(1, 0, 2)      # [co_in, co_ch, cls]
    arrs["fc"] = np.ascontiguousarray(fcT, np.float32)
    return arrs


def _apply_cn_affine(inputs):
    """The kernel folds ChannelNorm's (w, b) away assuming w==1, b==0
    (true for this model's initialization). Verify on host."""
    for k in KS:
        w = np.asarray(inputs[f"cn{k}_w"], np.float32)
        bb = np.asarray(inputs[f"cn{k}_b"], np.float32)
        if not (np.allclose(w, 1.0, atol=1e-6) and
                np.allclose(bb, 0.0, atol=1e-6)):
            raise NotImplementedError(
                "kernel assumes channel-norm weight==1, bias==0")


def kernel(run_opts=None, **inputs):
    _apply_cn_affine(inputs)
    nc = _get_nc()
    warrs = _prep_weights(inputs)
    x = np.ascontiguousarray(np.asarray(inputs["inputs"], np.float32))
    in_maps = []
    for c in range(NCORES):
        m = {"x": np.ascontiguousarray(x[c * BL:(c + 1) * BL])}
        m.update(warrs)
        in_maps.append(m)
    res = run_bass_kernel_spmd(nc, in_maps, core_ids=list(range(NCORES)),
                               **(run_opts or {}))
    out = np.concatenate([r["out"] for r in res.results], axis=0)
    if run_opts:
        return out, res
    return out



# revision 7
# speedup vs baseline: 1.3100x; 1.3100x over previous
"""Trainium2 Bass/Tile kernel for a 5-block 1D-CNN stack
(ChannelNorm -> ReLU -> Conv1d(k=4..8) -> sigmoid gate -> re-pad).

Data-parallel over batch: 32 samples -> 4 per NeuronCore x 8 cores.

v2 design ("PE does conv only"):
  * all activations bf16; conv weights bf16; matmuls at full PE rate
    (1 cycle/row) with fp32 PSUM accumulation.
  * layout-B tiles are stored as [s_in(128 part), chunk(2), t(8), c_in(128)]
    so both B->A and A->B layout changes run on the DMA xbar transpose
    engine (InstDmaTransposeAnt, out AP [128, 8, 128]:
    out[a, t, j] = in[j, t*128 + a]), freeing the PE entirely from
    transpose matmuls.
  * channel-norm rstd is computed on DVE with the int32 bit-trick +
    one Newton step, so the ACT engine only ever needs the
    {relu, sigmoid, copy} function table -> zero act-table reloads.
  * the sigmoid gate multiplier g = 1+score is folded algebraically into
    the next block's norm coefficients (scale = g*rstd',
    bias = -mean*scale, var scaled by g^2 -- exact); only the final
    block materializes the gated output.
"""

import numpy as np
import ml_dtypes

import concourse.bass as bass
import concourse.mybir as mybir
import concourse.tile as tile
from concourse import bacc
from concourse.bass_utils import run_bass_kernel_spmd

B, S, C = 32, 1024, 256
NCORES = 8
BL = B // NCORES  # batch per core
KS = [4, 5, 6, 7, 8]
PADS = {4: 1, 5: 2, 6: 2, 7: 3, 8: 3}
LS = {k: S + 2 * PADS[k] - k + 1 for k in KS}  # conv output lengths
EPS = 1e-5
UNB = float(C) / float(C - 1)  # unbiased-variance correction (ddof=1)
NT = S // 128  # 8 s-tiles of 128 per sample
F32 = mybir.dt.float32
BF16 = mybir.dt.bfloat16
I32 = mybir.dt.int32
MAGIC = 0x5F3759DF  # rsqrt bit-trick seed

_CACHED_NC = None


def _bn_stats_raw(nc, out, in_):
    """bn_stats with a strided multi-dim input treated as ONE group
    (out = 6 elems/partition). The bass wrapper would treat the extra
    input dim as a group dim; hardware reduces the whole pattern."""
    eng = nc.vector
    return eng.add_instruction(
        mybir.InstBNStats(
            name=nc.get_next_instruction_name(),
            ins=[eng.lower_ap(in_)],
            outs=[eng.lower_ap(out)],
        ))


def _build_block(nc, tc, pools, blk, k, xb_tiles, g_tile, w_sb, fc_sb,
                 out_dram):
    """Emit one conv block for all BL local batches.

    xb_tiles: list of BL SBUF tiles [128, 2, NT, 128] bf16, layout
    [s_in, c_chunk, t, c_in], holding the pre-gate block input.
    g_tile: None (block 0) or [128, BL, NT] f32 gate multiplier of the
    previous block. Returns (next_xb_tiles, g_next).
    """
    p = PADS[k]
    L = LS[k]
    last = (blk == len(KS) - 1)
    sm, xnb_p, xna_p, ha_p, psc, psy, xb_pool, out_p = pools

    g_next = sm.tile([128, BL, NT], F32, tag="g")
    next_xb = None if last else []

    for b in range(BL):
        xb_b = xb_tiles[b]

        # ---- channel-norm stats over C (strided view across both chunks) --
        bn6 = sm.tile([128, NT, 6], F32, tag="bn6")
        for t in range(NT):
            _bn_stats_raw(nc, bn6[:, t, :], xb_b[:, :, t, :])
        mv = sm.tile([128, NT, 2], F32, tag="mv")
        for t in range(NT):
            nc.vector.bn_aggr(out=mv[:, t, :], in_=bn6[:, t, :])
        mean = mv[:, :, 0:1].rearrange("p t o -> p (t o)")  # [128, NT]
        var = mv[:, :, 1:2].rearrange("p t o -> p (t o)")

        # ---- norm coefficients (gate of prev block folded in) ----
        if g_tile is None:
            zin = var
        else:
            g_b = g_tile[:, b, :]  # [128, NT]
            g2 = sm.tile([128, NT], F32, tag="g2")
            nc.vector.tensor_mul(g2, g_b, g_b)
            zin = sm.tile([128, NT], F32, tag="g2v")
            nc.vector.tensor_mul(zin, g2, var)
        # z = g^2 * var * UNB + eps
        z = sm.tile([128, NT], F32, tag="z")
        nc.vector.tensor_scalar(out=z, in0=zin, scalar1=UNB, scalar2=EPS,
                                op0=mybir.AluOpType.mult,
                                op1=mybir.AluOpType.add)
        # rr = rsqrt(z): int32 bit trick + one Newton step
        sh = sm.tile([128, NT], I32, tag="sh")
        nc.vector.tensor_scalar(out=sh, in0=z.bitcast(I32), scalar1=1,
                                scalar2=None,
                                op0=mybir.AluOpType.logical_shift_right)
        y0i = sm.tile([128, NT], I32, tag="y0i")
        nc.vector.tensor_scalar(out=y0i, in0=sh, scalar1=-1, scalar2=MAGIC,
                                op0=mybir.AluOpType.mult,
                                op1=mybir.AluOpType.add)
        y0 = y0i.bitcast(F32)
        t1 = sm.tile([128, NT], F32, tag="t1")
        nc.vector.tensor_mul(t1, y0, y0)
        nc.vector.tensor_mul(t1, t1, z)
        t3 = sm.tile([128, NT], F32, tag="t3")
        nc.vector.tensor_scalar(out=t3, in0=t1, scalar1=-0.5, scalar2=1.5,
                                op0=mybir.AluOpType.mult,
                                op1=mybir.AluOpType.add)
        rr = sm.tile([128, NT], F32, tag="rr")
        nc.vector.tensor_mul(rr, y0, t3)

        if g_tile is None:
            scale_c = rr
        else:
            scale_c = sm.tile([128, NT], F32, tag="scale_c")
            nc.vector.tensor_mul(scale_c, rr, g_tile[:, b, :])
        bias_c = sm.tile([128, NT], F32, tag="bias_c")
        # bias = -(mean * scale)
        nc.vector.scalar_tensor_tensor(out=bias_c, in0=mean, scalar=-1.0,
                                       in1=scale_c,
                                       op0=mybir.AluOpType.mult,
                                       op1=mybir.AluOpType.mult)

        # ---- fused normalize + relu (layout B) ----
        xnb = xnb_p.tile([128, 2, NT, 128], BF16, tag="xnB")
        for t in range(NT):
            nc.scalar.activation(out=xnb[:, :, t, :], in_=xb_b[:, :, t, :],
                                 func=mybir.ActivationFunctionType.Relu,
                                 scale=scale_c[:, t:t + 1],
                                 bias=bias_c[:, t:t + 1])

        # ---- B->A via one whole-tile DMA xbar transpose ----
        # in [s_in, (ch,t,ci)] -> out[ci, (ch,t), s_in] = xnA [ci, ch, s]
        xna = xna_p.tile([128, 2, S], BF16, tag="xnA")
        nc.sync.dma_start(
            out=xna.rearrange("p c (t s) -> p (c t) s", s=128),
            in_=xnb.rearrange("p c t s -> p (c t s)"), transpose=True)

        # ---- conv as matmuls (contract over ci x dk), layout A out ----
        ha = ha_p.tile([128, 2, S], BF16, tag="hA")
        dk_order = [p] + [d for d in range(k) if d != p]  # full-range first
        for co in range(2):
            for t2 in range(2):
                pc = psc.tile([128, 512], F32, tag="psc")
                idx = 0
                for ch in range(2):
                    for dk in dk_order:
                        sh2 = dk - p
                        a = t2 * 512 + sh2      # input window start
                        lo = max(0, -a)         # clip below 0
                        hi = 512 - max(0, a + 512 - S)  # clip above S
                        nc.tensor.matmul(
                            pc[:, lo:hi], w_sb[:, ch, co, dk, :],
                            xna[:, ch, a + lo:a + hi],
                            start=(idx == 0), stop=(idx == 2 * k - 1))
                        idx += 1
                dst = ha[:, co, t2 * 512:(t2 + 1) * 512]
                if t2 == 0:
                    nc.scalar.copy(out=dst, in_=pc)
                else:
                    nc.vector.tensor_copy(out=dst, in_=pc)
        if L < S:  # zero the re-pad tail
            nc.gpsimd.memset(ha[:, :, L:S], 0.0)

        # ---- gate: y^T[s, cls] = h^T @ fc, all 8 s-tiles in one bank ----
        py_t = psy.tile([128, NT, 2], F32, tag="psy")
        for st in range(NT):
            for co in range(2):
                nc.tensor.matmul(py_t[:, st, :],
                                 ha[:, co, st * 128:(st + 1) * 128],
                                 fc_sb[:, co, :],
                                 start=(co == 0), stop=(co == 1))
        ya2 = sm.tile([128, NT, 2], F32, tag="ya2")
        nc.vector.tensor_copy(out=ya2, in_=py_t)
        ya = ya2[:, :, 0:1].rearrange("p t o -> p (t o)")  # [128, NT]
        yb = ya2[:, :, 1:2].rearrange("p t o -> p (t o)")
        smax = sm.tile([128, NT], F32, tag="smax")
        nc.vector.tensor_max(smax, ya, yb)
        ssum = sm.tile([128, NT], F32, tag="ssum")
        nc.vector.tensor_add(ssum, ya, yb)
        pre = sm.tile([128, NT], F32, tag="pre")
        # pre = 0.2*max + (ya+yb);  score = sigmoid(0.5*pre)
        nc.vector.scalar_tensor_tensor(out=pre, in0=smax, scalar=0.2,
                                       in1=ssum,
                                       op0=mybir.AluOpType.mult,
                                       op1=mybir.AluOpType.add)
        sc = sm.tile([128, NT], F32, tag="sc")
        nc.scalar.activation(out=sc, in_=pre,
                             func=mybir.ActivationFunctionType.Sigmoid,
                             scale=0.5)
        nc.vector.tensor_scalar_add(g_next[:, b, :], sc, 1.0)

        # ---- A->B via one whole-tile DMA xbar transpose ----
        # in [co, (ch? s)] flat -> out [s_in, (ch,t), co] = xb layout
        xb2 = xb_pool.tile([128, 2, NT, 128], BF16, tag="xB")
        nc.sync.dma_start(
            out=xb2.rearrange("p c t s -> p (c t) s"),
            in_=ha.rearrange("p c s -> p (c s)"), transpose=True)
        if last:
            # materialize gated output in f32, [s_in, t, c] for the out DMA
            ot = out_p.tile([128, NT, C], F32, tag="osb")
            for t in range(NT):
                nc.gpsimd.tensor_scalar_mul(
                    out=ot[:, t, :].rearrange("p (a b) -> p a b", a=2),
                    in0=xb2[:, :, t, :],
                    scalar1=g_next[:, b, t:t + 1])
            dst = out_dram[b].rearrange("(t p) c -> p t c", p=128)
            nc.sync.dma_start(out=dst, in_=ot)
        else:
            next_xb.append(xb2)

    return next_xb, g_next


def _build():
    nc = bacc.Bacc("TRN2", target_bir_lowering=False, debug=False,
                   num_devices=NCORES)
    x_in = nc.dram_tensor("x", [BL, 128, 2, NT, 128], BF16,
                          kind="ExternalInput").ap()
    w_in = {k: nc.dram_tensor(f"w{k}", [128, 2, 2, k, 128], BF16,
                              kind="ExternalInput").ap() for k in KS}
    fc_in = nc.dram_tensor("fc", [128, 2, 2], BF16,
                           kind="ExternalInput").ap()
    out_dram = nc.dram_tensor("out", [BL, S, C], F32,
                              kind="ExternalOutput").ap()

    from contextlib import ExitStack
    with tile.TileContext(nc) as tc, ExitStack() as ctx:
        consts = ctx.enter_context(tc.tile_pool(name="consts", bufs=1))
        wpool = ctx.enter_context(tc.tile_pool(name="wpool", bufs=1))
        xb_pool = ctx.enter_context(tc.tile_pool(name="xb", bufs=8))
        xnb_p = ctx.enter_context(tc.tile_pool(name="xnb", bufs=4))
        xna_p = ctx.enter_context(tc.tile_pool(name="xna", bufs=4))
        ha_p = ctx.enter_context(tc.tile_pool(name="ha", bufs=4))
        sm = ctx.enter_context(tc.tile_pool(name="small", bufs=8))
        out_p = ctx.enter_context(tc.tile_pool(name="outp", bufs=2))
        psc = ctx.enter_context(tc.tile_pool(name="psc", bufs=6,
                                             space="PSUM"))
        psy = ctx.enter_context(tc.tile_pool(name="psy", bufs=2,
                                             space="PSUM"))

        fc_sb = consts.tile([128, 2, 2], BF16)
        nc.sync.dma_start(out=fc_sb, in_=fc_in)

        # prefetch all conv weights
        w_tiles = {}
        for k in KS:
            w_sb = wpool.tile([128, 2, 2, k, 128], BF16, tag=f"w{k}")
            nc.sync.dma_start(out=w_sb, in_=w_in[k])
            w_tiles[k] = w_sb

        # initial load: host-packed layout-B bf16 tiles
        xb_tiles = []
        for b in range(BL):
            t = xb_pool.tile([128, 2, NT, 128], BF16, tag="xB")
            nc.sync.dma_start(out=t, in_=x_in[b])
            xb_tiles.append(t)

        pools = (sm, xnb_p, xna_p, ha_p, psc, psy, xb_pool, out_p)
        g_tile = None
        for blk, k in enumerate(KS):
            xb_tiles, g_tile = _build_block(
                nc, tc, pools, blk, k, xb_tiles, g_tile, w_tiles[k],
                fc_sb, out_dram)

    nc.compile()
    return nc


def _get_nc():
    global _CACHED_NC
    if _CACHED_NC is None:
        _CACHED_NC = _build()
    return _CACHED_NC


def _prep_weights(inputs):
    """Host-side packing of conv / fc weights into the DRAM layouts."""
    arrs = {}
    for k in KS:
        W = np.asarray(inputs[f"W{k}"], np.float32)  # [co, ci, k]
        Wt = W.transpose(1, 0, 2)                    # [ci, co, k]
        Wt = Wt.reshape(2, 128, 2, 128, k)           # [ci_ch, ci_in, co_ch, co_in, k]
        Wt = Wt.transpose(1, 0, 2, 4, 3)             # [ci_in, ci_ch, co_ch, k, co_in]
        arrs[f"w{k}"] = np.ascontiguousarray(Wt).astype(ml_dtypes.bfloat16)
    fc = np.asarray(inputs["fc_w"], np.float32)[:, :, 0]  # [2, co=256]
    fcT = fc.T.reshape(2, 128, 2).transpose(1, 0, 2)      # [co_in, co_ch, cls]
    arrs["fc"] = np.ascontiguousarray(fcT).astype(ml_dtypes.bfloat16)
    return arrs


def _check_cn_affine(inputs):
    """The kernel folds ChannelNorm's (w, b) away assuming w==1, b==0
    (true for this model's initialization). Verify on host."""
    for k in KS:
        w = np.asarray(inputs[f"cn{k}_w"], np.float32)
        bb = np.asarray(inputs[f"cn{k}_b"], np.float32)
        if not (np.allclose(w, 1.0, atol=1e-6) and
                np.allclose(bb, 0.0, atol=1e-6)):
            raise NotImplementedError(
                "kernel assumes channel-norm weight==1, bias==0")


def _pack_x(x):
    # [BL, S, C] f32 -> [BL, s_in, ch, t, ci] bf16
    xr = x.reshape(-1, NT, 128, 2, 128)      # [b, t, s_in, ch, ci]
    xt = xr.transpose(0, 2, 3, 1, 4)         # [b, s_in, ch, t, ci]
    return np.ascontiguousarray(xt).astype(ml_dtypes.bfloat16)


def kernel(run_opts=None, **inputs):
    _check_cn_affine(inputs)
    nc = _get_nc()
    warrs = _prep_weights(inputs)
    x = np.ascontiguousarray(np.asarray(inputs["inputs"], np.float32))
    in_maps = []
    for c in range(NCORES):
        m = {"x": _pack_x(x[c * BL:(c + 1) * BL])}
        m.update(warrs)
        in_maps.append(m)
    res = run_bass_kernel_spmd(nc, in_maps, core_ids=list(range(NCORES)),
                               **(run_opts or {}))
    out = np.concatenate([r["out"] for r in res.results], axis=0)
    if run_opts:
        return out, res
    return out
